# revision 4
# baseline (speedup 1.0000x reference)
"""Masked dot-product attention on 8 Trainium2 NeuronCores (Bass/Tile).

Problem: B=8, Nq=2048, Nk=2048, D=64 fp32; per-batch valid_lens masks keys
k >= L with -1e6 before softmax (== excluding them).

Sharding: data-parallel over batch; core b handles batch b.

Per-core algorithm (all layouts prepared on host):
  S^T[k, q] = K[k, :] . Q[q, :]                (PE, f32r, row-packed pairs of
                                                64-contraction matmuls)
  E^T = exp(S^T / 8)                           (ACT, fused scale; no max
                                                subtraction: |s|/8 <= ~6)
  U^T[d', q] = sum_k V'[k, d'] E^T[k, q]       (PE accumulation over chunks)
     where V' = [V | ones], with rows k >= L zeroed on host: masked keys
     contribute 0 to both numerator and denominator, so no mask is needed
     on-device. U^T row 64 is the softmax denominator.
  O[q, d] = U^T[d, q] / U^T[64, q]             (PE transpose + DVE reciprocal
                                                + per-partition scale)

Numerics: matmuls in float32r (~1.5e-4 rms rel rounding), exp on ACT
(~1e-5), accumulation fp32 in PSUM. No-max softmax is exact in this score
range (exp <= e^6).
"""
import os
import sys

for _p in ("/opt/trn_rl_repo", "/root/.axon_site/_ro/trn_rl_repo"):
    if os.path.isdir(_p):
        if _p not in sys.path:
            sys.path.insert(0, _p)
        break

import ml_dtypes
import numpy as np

import bass_rust
import concourse.bass as bass
import concourse.tile as tile
from concourse import mybir
from concourse.bass_utils import run_bass_kernel_spmd

F32 = mybir.dt.float32
F32R = mybir.dt.float32r
BF16 = mybir.dt.bfloat16

B, NQ, NK, D = 8, 2048, 2048, 64
NCHUNK = NK // 128          # 16 key chunks of 128
NPAIR = NCHUNK // 2         # 8 chunk pairs (row-packed matmuls)
QG = 512                    # q columns per moving pass
NQG = NQ // QG              # 4 q groups
VW = D + 1                  # V' width (ones column appended)

# f32r input blob layout (free dim): K2 | QT_dup
OFF_K2 = 0
OFF_QT = OFF_K2 + NCHUNK * 64
BLOB_W = OFF_QT + NQ
VBLOB_W = NCHUNK * VW
SPLIT_A = OFF_QT + QG  # first DMA: K2 + QT[qg0]


def _split_waits(nc, maxw=1):
    """Walrus in this container rejects >1 sync wait on many instruction
    structs; hoist excess waits onto NoOps inserted just before."""
    cnt = 0
    for f in nc.m.functions:
        for bb in f.blocks:
            insts = bb.instructions
            i = 0
            while i < len(insts):
                ins = insts[i]
                si = ins.sync_info
                waits = list(si.on_wait) if si is not None and si.on_wait else []
                if len(waits) > maxw:
                    keep = waits[len(waits) - maxw:]
                    excess = waits[: len(waits) - maxw]
                    for j in range(0, len(excess), maxw):
                        cnt += 1
                        nop = mybir.InstNoOp(name=f"I-ws{cnt}", ins=[], outs=[])
                        nop.engine = ins.engine
                        nop.sync_info = bass_rust.SyncInfo(
                            on_wait=excess[j : j + maxw], on_update=[]
                        )
                        insts.insert(i, nop)
                        i += 1
                    ins.sync_info = bass_rust.SyncInfo(
                        on_wait=keep, on_update=list(si.on_update or [])
                    )
                i += 1
    return cnt


_BUILT = None


def _build():
    nc = bass.Bass(trn_type="TRN2")
    br = nc.dram_tensor("br", [128, BLOB_W], F32R, kind="ExternalInput")
    bv = nc.dram_tensor("bv", [128, VBLOB_W], BF16, kind="ExternalInput")
    bident = nc.dram_tensor("bident", [128, 128], F32, kind="ExternalInput")
    o = nc.dram_tensor("o", [NQ, D], F32, kind="ExternalOutput")
    o_tiled = o.rearrange("(c p) d -> p c d", p=128)  # [128, 16, 64]

    with tile.TileContext(nc) as tc:
        with (
            tc.tile_pool(name="consts", bufs=1) as consts,
            tc.tile_pool(name="epool", bufs=3) as epool,
            tc.tile_pool(name="usb", bufs=2) as usb,
            tc.tile_pool(name="rpool", bufs=2) as rpool,
            tc.tile_pool(name="s2pool", bufs=2, space="PSUM") as s2pool,
            tc.tile_pool(name="upool", bufs=2, space="PSUM") as upool,
            tc.tile_pool(name="opool", bufs=2, space="PSUM") as opool,
        ):
            tbr = consts.tile([128, BLOB_W], F32R)
            nc.sync.dma_start(tbr[:, 0:SPLIT_A], br[:, 0:SPLIT_A])
            tbv = consts.tile([128, NCHUNK, VW], BF16)
            nc.sync.dma_start(tbv[:], bv[:])
            nc.sync.dma_start(tbr[:, SPLIT_A:BLOB_W], br[:, SPLIT_A:BLOB_W])
            tid = consts.tile([128, 128], F32)
            nc.sync.dma_start(tid[:], bident[:])
            tout = consts.tile([128, NCHUNK, D], F32)

            qt = tbr[:, OFF_QT : OFF_QT + NQ]
            k2 = tbr[:, OFF_K2 : OFF_QT]
            v1 = tbv

            for qg in range(NQG):
                qs = slice(qg * QG, (qg + 1) * QG)
                ut = upool.tile([VW, QG], F32)
                for pj in range(NPAIR):
                    ks = slice(pj * 128, (pj + 1) * 128)
                    s2 = s2pool.tile([128, 2, QG], F32)
                    nc.tensor.matmul(s2[:, 0, :], k2[0:64, ks], qt[0:64, qs],
                                     start=True, stop=True, tile_position=(0, 0))
                    nc.tensor.matmul(s2[:, 1, :], k2[64:128, ks], qt[64:128, qs],
                                     start=True, stop=True, tile_position=(64, 0))
                    e2 = epool.tile([128, 2, QG], BF16)
                    nc.scalar.activation(e2[:, :, :], s2[:, :, :],
                                         mybir.ActivationFunctionType.Exp,
                                         scale=0.125)
                    nc.tensor.matmul(ut[:], v1[:, 2 * pj, :], e2[:, 0, :],
                                     start=(pj == 0), stop=False)
                    nc.tensor.matmul(ut[:], v1[:, 2 * pj + 1, :], e2[:, 1, :],
                                     start=False, stop=(pj == NPAIR - 1))

                uts = usb.tile([VW, QG], F32)
                nc.vector.tensor_copy(uts[:], ut[:])
                for c in range(4):
                    chunk = qg * 4 + c
                    ot = opool.tile([128, VW], F32)
                    nc.tensor.transpose(ot[:], uts[:, c * 128 : (c + 1) * 128],
                                        tid[0:VW, 0:VW])
                    rec = rpool.tile([128, 1], F32)
                    nc.vector.reciprocal(rec[:], ot[:, D : D + 1])
                    nc.vector.tensor_scalar_mul(tout[:, chunk, :], ot[:, 0:D],
                                                rec[:])
                nc.sync.dma_start(o_tiled[:, qg * 4 : (qg + 1) * 4, :],
                                  tout[:, qg * 4 : (qg + 1) * 4, :])

    _split_waits(nc)
    return nc


def _host_prep(queries, keys, values, valid_lens):
    """Per-core f32r blobs: [QT duplicated onto both partition halves |
    K^T chunk pairs split across partition halves | V' chunk-packed]."""
    queries = np.asarray(queries, dtype=np.float32)
    keys = np.asarray(keys, dtype=np.float32)
    values = np.asarray(values, dtype=np.float32)
    valid_lens = np.asarray(valid_lens)

    in_maps = []
    ident = np.eye(128, dtype=np.float32)
    for b in range(B):
        blob = np.empty((128, BLOB_W), dtype=np.float32)
        qt = queries[b].T  # [64, 2048]
        blob[0:64, OFF_QT : OFF_QT + NQ] = qt
        blob[64:128, OFF_QT : OFF_QT + NQ] = qt
        kt = keys[b].T  # [64, 2048]
        k2 = kt.reshape(64, NPAIR, 2, 128).transpose(2, 0, 1, 3).reshape(128, NPAIR * 128)
        blob[:, OFF_K2 : OFF_QT] = k2
        vp = np.concatenate(
            [values[b], np.ones((NK, 1), dtype=np.float32)], axis=1
        )  # [2048, 65]
        vp[int(valid_lens[b]):, :] = 0.0
        vblob = (
            vp.reshape(NCHUNK, 128, VW).transpose(1, 0, 2).reshape(128, NCHUNK * VW)
        ).astype(ml_dtypes.bfloat16)
        in_maps.append({"br": blob, "bv": vblob, "bident": ident})
    return in_maps


def kernel(queries, keys, values, valid_lens):
    global _BUILT
    if _BUILT is None:
        _BUILT = _build()
    in_maps = _host_prep(queries, keys, values, valid_lens)
    res = run_bass_kernel_spmd(
        _BUILT,
        in_maps,
        core_ids=list(range(B)),
        trace=bool(os.environ.get("KERNEL_TRACE")),
    )
    kernel.last_result = res
    out = np.stack([np.asarray(res.results[b]["o"]) for b in range(B)], axis=0)
    return out.astype(np.float32)


# revision 5
# speedup vs baseline: 1.1421x; 1.1421x over previous
"""Masked dot-product attention on 8 Trainium2 NeuronCores (Bass/Tile).

Problem: B=8, Nq=2048, Nk=2048, D=64 fp32; per-batch valid_lens masks keys
k >= L with -1e6 before softmax (== excluding them).

Sharding: data-parallel over batch; core b handles batch b.

Per-core algorithm (all layouts prepared on host):
  S^T[k, q] = K[k, :] . Q[q, :]                (PE, f32r, row-packed pairs of
                                                64-contraction matmuls)
  E^T = exp(S^T / 8)                           (ACT, fused scale; no max
                                                subtraction: |s|/8 <= ~6)
  U^T[d', q] = sum_k V'[k, d'] E^T[k, q]       (PE accumulation over chunks)
     where V' = [V | ones], with rows k >= L zeroed on host: masked keys
     contribute 0 to both numerator and denominator, so no mask is needed
     on-device. U^T row 64 is the softmax denominator.
  O[q, d] = U^T[d, q] / U^T[64, q]             (PE transpose + DVE reciprocal
                                                + per-partition scale)

Numerics: matmuls in float32r (~1.5e-4 rms rel rounding), exp on ACT
(~1e-5), accumulation fp32 in PSUM. No-max softmax is exact in this score
range (exp <= e^6).
"""
import os
import sys

for _p in ("/opt/trn_rl_repo", "/root/.axon_site/_ro/trn_rl_repo"):
    if os.path.isdir(_p):
        if _p not in sys.path:
            sys.path.insert(0, _p)
        break

import ml_dtypes
import numpy as np

import bass_rust
import concourse.bass as bass
import concourse.tile as tile
from concourse import mybir
from concourse.bass_utils import run_bass_kernel_spmd

F32 = mybir.dt.float32
F32R = mybir.dt.float32r
BF16 = mybir.dt.bfloat16

B, NQ, NK, D = 8, 2048, 2048, 64
NCHUNK = NK // 128          # 16 key chunks of 128
NPAIR = NCHUNK // 2         # 8 chunk pairs (row-packed matmuls)
QG = 512                    # q columns per moving pass
NQG = NQ // QG              # 4 q groups
VW = D + 1                  # V' width (ones column appended)

# f32r input blob layout (free dim): K2 | QT_dup
OFF_K2 = 0
OFF_QT = OFF_K2 + NCHUNK * 64
BLOB_W = OFF_QT + NQ
VBLOB_W = NCHUNK * VW
SPLIT_A = OFF_QT + QG  # first DMA: K2 + QT[qg0]


def _split_waits(nc, maxw=1):
    """Walrus in this container rejects >1 sync wait on many instruction
    structs; hoist excess waits onto NoOps inserted just before."""
    cnt = 0
    for f in nc.m.functions:
        for bb in f.blocks:
            insts = bb.instructions
            i = 0
            while i < len(insts):
                ins = insts[i]
                si = ins.sync_info
                waits = list(si.on_wait) if si is not None and si.on_wait else []
                if len(waits) > maxw:
                    keep = waits[len(waits) - maxw:]
                    excess = waits[: len(waits) - maxw]
                    for j in range(0, len(excess), maxw):
                        cnt += 1
                        nop = mybir.InstNoOp(name=f"I-ws{cnt}", ins=[], outs=[])
                        nop.engine = ins.engine
                        nop.sync_info = bass_rust.SyncInfo(
                            on_wait=excess[j : j + maxw], on_update=[]
                        )
                        insts.insert(i, nop)
                        i += 1
                    ins.sync_info = bass_rust.SyncInfo(
                        on_wait=keep, on_update=list(si.on_update or [])
                    )
                i += 1
    return cnt


_BUILT = None


def _build():
    nc = bass.Bass(trn_type="TRN2")
    br = nc.dram_tensor("br", [128, BLOB_W], F32R, kind="ExternalInput")
    bv = nc.dram_tensor("bv", [128, VBLOB_W], BF16, kind="ExternalInput")
    bident = nc.dram_tensor("bident", [128, 128], F32, kind="ExternalInput")
    o = nc.dram_tensor("o", [NQ, D], F32, kind="ExternalOutput")
    o_tiled = o.rearrange("(c p) d -> p c d", p=128)  # [128, 16, 64]

    with tile.TileContext(nc) as tc:
        with (
            tc.tile_pool(name="consts", bufs=1) as consts,
            tc.tile_pool(name="epool", bufs=3) as epool,
            tc.tile_pool(name="usb", bufs=2) as usb,
            tc.tile_pool(name="rpool", bufs=2) as rpool,
            tc.tile_pool(name="s2pool", bufs=2, space="PSUM") as s2pool,
            tc.tile_pool(name="upool", bufs=2, space="PSUM") as upool,
            tc.tile_pool(name="opool", bufs=2, space="PSUM") as opool,
        ):
            tbrA = consts.tile([128, SPLIT_A], F32R)
            nc.sync.dma_start(tbrA[:], br[:, 0:SPLIT_A])
            tbv = consts.tile([128, NCHUNK, VW], BF16)
            nc.sync.dma_start(tbv[:], bv[:])
            tbrB = consts.tile([128, BLOB_W - SPLIT_A], F32R)
            nc.sync.dma_start(tbrB[:], br[:, SPLIT_A:BLOB_W])
            tid = consts.tile([128, 128], F32)
            nc.sync.dma_start(tid[:], bident[:])
            tout = consts.tile([128, NCHUNK, D], F32)

            def qt_slice(qg):
                if qg == 0:
                    return tbrA[:, OFF_QT : OFF_QT + QG]
                return tbrB[:, (qg - 1) * QG : qg * QG]

            k2 = tbrA[:, OFF_K2 : OFF_QT]
            v1 = tbv

            for qg in range(NQG):
                qtg = qt_slice(qg)
                ut = upool.tile([VW, QG], F32)
                for pj in range(NPAIR):
                    ks = slice(pj * 128, (pj + 1) * 128)
                    s2 = s2pool.tile([128, 2, QG], F32)
                    nc.tensor.matmul(s2[:, 0, :], k2[0:64, ks], qtg[0:64, :],
                                     start=True, stop=True, tile_position=(0, 0))
                    nc.tensor.matmul(s2[:, 1, :], k2[64:128, ks], qtg[64:128, :],
                                     start=True, stop=True, tile_position=(64, 0))
                    e2 = epool.tile([128, 2, QG], BF16)
                    nc.scalar.activation(e2[:, :, :], s2[:, :, :],
                                         mybir.ActivationFunctionType.Exp,
                                         scale=0.125)
                    nc.tensor.matmul(ut[:], v1[:, 2 * pj, :], e2[:, 0, :],
                                     start=(pj == 0), stop=False)
                    nc.tensor.matmul(ut[:], v1[:, 2 * pj + 1, :], e2[:, 1, :],
                                     start=False, stop=(pj == NPAIR - 1))

                uts = usb.tile([VW, QG], F32)
                nc.vector.tensor_copy(uts[:], ut[:])
                for c in range(4):
                    chunk = qg * 4 + c
                    ot = opool.tile([128, VW], F32)
                    nc.tensor.transpose(ot[:], uts[:, c * 128 : (c + 1) * 128],
                                        tid[0:VW, 0:VW])
                    rec = rpool.tile([128, 1], F32)
                    nc.vector.reciprocal(rec[:], ot[:, D : D + 1])
                    nc.vector.tensor_scalar_mul(tout[:, chunk, :], ot[:, 0:D],
                                                rec[:])
                nc.sync.dma_start(o_tiled[:, qg * 4 : (qg + 1) * 4, :],
                                  tout[:, qg * 4 : (qg + 1) * 4, :])

    _split_waits(nc)
    return nc


def _host_prep(queries, keys, values, valid_lens):
    """Per-core f32r blobs: [QT duplicated onto both partition halves |
    K^T chunk pairs split across partition halves | V' chunk-packed]."""
    queries = np.asarray(queries, dtype=np.float32)
    keys = np.asarray(keys, dtype=np.float32)
    values = np.asarray(values, dtype=np.float32)
    valid_lens = np.asarray(valid_lens)

    in_maps = []
    ident = np.eye(128, dtype=np.float32)
    for b in range(B):
        blob = np.empty((128, BLOB_W), dtype=np.float32)
        qt = queries[b].T  # [64, 2048]
        blob[0:64, OFF_QT : OFF_QT + NQ] = qt
        blob[64:128, OFF_QT : OFF_QT + NQ] = qt
        kt = keys[b].T  # [64, 2048]
        k2 = kt.reshape(64, NPAIR, 2, 128).transpose(2, 0, 1, 3).reshape(128, NPAIR * 128)
        blob[:, OFF_K2 : OFF_QT] = k2
        vp = np.concatenate(
            [values[b], np.ones((NK, 1), dtype=np.float32)], axis=1
        )  # [2048, 65]
        vp[int(valid_lens[b]):, :] = 0.0
        vblob = (
            vp.reshape(NCHUNK, 128, VW).transpose(1, 0, 2).reshape(128, NCHUNK * VW)
        ).astype(ml_dtypes.bfloat16)
        in_maps.append({"br": blob, "bv": vblob, "bident": ident})
    return in_maps


def kernel(queries, keys, values, valid_lens):
    global _BUILT
    if _BUILT is None:
        _BUILT = _build()
    in_maps = _host_prep(queries, keys, values, valid_lens)
    res = run_bass_kernel_spmd(
        _BUILT,
        in_maps,
        core_ids=list(range(B)),
        trace=bool(os.environ.get("KERNEL_TRACE")),
    )
    kernel.last_result = res
    out = np.stack([np.asarray(res.results[b]["o"]) for b in range(B)], axis=0)
    return out.astype(np.float32)
